# revision 16
# baseline (speedup 1.0000x reference)
"""2-layer GCN on 8 Trainium2 NeuronCores — fp8(e3m4) tables + grouped gathers.

Strategy (nodes partitioned across 8 cores, graph/data parallel):
  - x@W1 with host-pretransposed fp8 x tiles (feature dim on partitions),
    W1 in bf16 (mixed-dtype matmul), fp32 PSUM accum.
  - Node tables (h, z) stored fp8 e3m4 with power-of-2 scaling (x*2 -> h'=2h,
    norm*4 -> z'=8z out of the relu, W2/32 compensates exactly) so table
    values sit in e3m4's normal range.
  - Edges sharded by dst core, grouped by 128-node dst block, split lo/hi
    src ranges (int16 dma_gather limit); per GROUP of GS dst blocks one
    lo-gather + one hi-gather (256B rows, GS=3, 3 groups in flight) to
    amortize SWDGE/DMA overheads and deepen the gather/compute pipeline.
  - Segment-sum via one-hot matrix M[e,d] = (iota==dstloc)*norm built in one
    DVE op (bf16), PE matmul accumulation into PSUM.
  - Self loops added per dst block from the local table (no gather).
  - Layer 2 reuses the same machinery on z', then out = agg2 @ (W2/32) + b2.
"""
import sys
sys.path.insert(0, "/opt/trn_rl_repo")
import os
import numpy as np
import ml_dtypes

N = 50000
NPAD = 50176
NC = 8
PC = NPAD // NC          # 6272 nodes per core
B = PC // 128            # 49 dst blocks per core
KDIM = 7688
KPAD = 7808
KO = KPAD // 128         # 61
H = 200
HPAD = 256
O = 8
LO = 32768               # src < LO -> lo table view [0:32768)
HIB = NPAD - 32768       # 17408; hi view rows [HIB:NPAD), idx' = src - HIB

BF16 = ml_dtypes.bfloat16
F8 = ml_dtypes.float8_e3m4
SINGLE_PKT = os.environ.get("SINGLE_PKT", "0") == "1"


def _preprocess(x, edge_weight, W1, b1, W2, b2, edge_index, gs=None):
    GS = gs if gs is not None else int(os.environ.get("GS", "3"))
    x = np.asarray(x, dtype=np.float32)
    edge_weight = np.asarray(edge_weight, dtype=np.float32)
    W1 = np.asarray(W1, dtype=np.float32)
    b1 = np.asarray(b1, dtype=np.float32)
    W2 = np.asarray(W2, dtype=np.float32)
    b2 = np.asarray(b2, dtype=np.float32)
    src = np.asarray(edge_index[0], dtype=np.int64)
    dst = np.asarray(edge_index[1], dtype=np.int64)

    # --- gcn_norm (self loops, symmetric normalization) + fp8 scaling
    deg = np.bincount(dst, weights=edge_weight.astype(np.float64), minlength=N)
    deg += 1.0
    dis = np.where(deg > 0, deg ** -0.5, 0.0).astype(np.float32)
    norm = 4.0 * dis[src] * edge_weight * dis[dst]
    dis2 = 4.0 * (dis * dis).astype(np.float32)

    # --- shard edges by dst core, group by dst block, split lo/hi by src
    core = dst // PC
    block = (dst % PC) // 128
    dstloc128 = (dst % PC) % 128
    lohi = (src >= LO).astype(np.int64)

    key = (core * B + block) * 2 + lohi
    order = np.argsort(key, kind="stable")
    src_s, norm_s, dl_s, key_s = src[order], norm[order], dstloc128[order], key[order]
    counts = np.bincount(key_s, minlength=NC * B * 2).reshape(NC, B, 2)
    starts = np.zeros(NC * B * 2 + 1, dtype=np.int64)
    np.cumsum(counts.ravel(), out=starts[1:])

    # tiles per (block, stream): max over cores (single SPMD program)
    T_LO = np.maximum(1, -(-counts[:, :, 0].max(axis=0) // 128))   # [B]
    T_HI = np.maximum(1, -(-counts[:, :, 1].max(axis=0) // 128))

    # --- grouped column layout: per group, all LO tiles, all HI tiles, then
    # one self-loop slot per block (dl=iota, nm=dis2) accumulated like a tile.
    groups = [list(range(g, min(g + GS, B))) for g in range(0, B, GS)]
    g_col0, g_lo, g_hi = [], [], []
    lo_off = np.zeros(B, dtype=np.int64)
    hi_off = np.zeros(B, dtype=np.int64)
    self_off = np.zeros(B, dtype=np.int64)
    col = 0
    for grp in groups:
        g_col0.append(col)
        glo = int(sum(T_LO[b] for b in grp))
        ghi = int(sum(T_HI[b] for b in grp))
        g_lo.append(glo); g_hi.append(ghi)
        c = col
        for b in grp:
            lo_off[b] = c; c += T_LO[b]
        for b in grp:
            hi_off[b] = c; c += T_HI[b]
        for b in grp:
            self_off[b] = c; c += 1
        col += glo + ghi + len(grp)
    TSUM = col
    GTMAX = max(g_lo[i] + g_hi[i] + len(groups[i]) for i in range(len(groups)))

    # --- per-core arrays
    per_core = []
    x_pad = np.zeros((NPAD, KPAD), dtype=np.float32)
    x_pad[:N, :KDIM] = 2.0 * x
    W1_pre = np.zeros((KPAD, H), dtype=np.float32)
    W1_pre[:KDIM] = W1
    W1_pre = np.ascontiguousarray(
        W1_pre.reshape(KO, 128, H).transpose(1, 0, 2)).astype(BF16)
    b1_rep = np.tile(8.0 * b1[None, :], (128, 1)).astype(np.float32)
    b2_rep = np.tile(b2[None, :], (128, 1)).astype(np.float32)
    W2_pre = np.zeros((128, 2, O), dtype=np.float32)
    W2_pre[:128, 0, :] = W2[:128] / 32.0
    W2_pre[: H - 128, 1, :] = W2[128:H] / 32.0
    iota_row = np.tile(np.arange(128, dtype=np.float32), (128, 1)).astype(BF16)
    dis2_pad = np.zeros(NPAD, dtype=np.float32)
    dis2_pad[:N] = dis2

    for r in range(NC):
        xr = x_pad[r * PC:(r + 1) * PC]
        x_pre = np.ascontiguousarray(
            xr.reshape(B, 128, KO, 128).transpose(0, 3, 2, 1)
        ).reshape(B, 128, KPAD).astype(F8)

        idx_flat = np.zeros((TSUM * 128,), dtype=np.int64)
        dl_flat = np.zeros((TSUM * 128,), dtype=np.float32)
        nm_flat = np.zeros((TSUM * 128,), dtype=np.float32)
        for b in range(B):
            for s, off, Tn in ((0, lo_off[b], int(T_LO[b])),
                               (1, hi_off[b], int(T_HI[b]))):
                gkey = (r * B + b) * 2 + s
                lo_i, hi_i = starts[gkey], starts[gkey + 1]
                n = hi_i - lo_i
                o0 = off * 128
                ids = src_s[lo_i:hi_i] - (0 if s == 0 else HIB)
                idx_flat[o0:o0 + n] = ids
                dl_flat[o0:o0 + n] = dl_s[lo_i:hi_i]
                nm_flat[o0:o0 + n] = norm_s[lo_i:hi_i]
            so = self_off[b] * 128
            dl_flat[so:so + 128] = np.arange(128, dtype=np.float32)
            nm_flat[so:so + 128] = dis2_pad[r * PC + b * 128:
                                            r * PC + (b + 1) * 128]
        # wrap idx per contiguous gather range into [128, cols] int16
        idx_cols = np.zeros((128, TSUM * 8), dtype=np.int16)
        for gi, grp in enumerate(groups):
            for c0, Tn in ((g_col0[gi], g_lo[gi]),
                           (g_col0[gi] + g_lo[gi], g_hi[gi])):
                grp_idx = idx_flat[c0 * 128:(c0 + Tn) * 128].astype(np.int16)
                w = np.tile(grp_idx.reshape(-1, 16).T, (8, 1))   # [128, Tn*8]
                idx_cols[:, c0 * 8:(c0 + Tn) * 8] = w
        dstloc = dl_flat.reshape(TSUM, 128).T.copy()
        normv = nm_flat.reshape(TSUM, 128).T.copy()
        dis2_blk = dis2_pad[r * PC:(r + 1) * PC].reshape(B, 128).T.copy()

        per_core.append({
            "x_pre": x_pre, "w1": W1_pre, "b1r": b1_rep, "b2r": b2_rep,
            "w2": W2_pre, "iota": iota_row, "idx": idx_cols,
            "dstloc": dstloc, "normv": normv, "dis2": dis2_blk,
        })

    meta = {
        "b1_zero": bool(not np.any(b1)),
        "b2_zero": bool(not np.any(b2)),
        "T_LO": [int(v) for v in T_LO],
        "T_HI": [int(v) for v in T_HI],
        "lo_off": [int(v) for v in lo_off],
        "hi_off": [int(v) for v in hi_off],
        "self_off": [int(v) for v in self_off],
        "groups": groups,
        "g_col0": g_col0, "g_lo": g_lo, "g_hi": g_hi,
        "TSUM": TSUM, "GTMAX": GTMAX,
    }
    return per_core, meta


def _scope(nc, name, it):
    with nc.named_scope(name):
        yield from it


def _build_program(meta, sim_mode=False):
    import concourse.bass as bass
    import concourse.bacc as bacc
    import concourse.mybir as mybir
    import concourse.tile as tile
    from concourse.masks import make_identity

    T_LO, T_HI = meta["T_LO"], meta["T_HI"]
    lo_off, hi_off = meta["lo_off"], meta["hi_off"]
    self_off = meta["self_off"]
    groups, g_col0 = meta["groups"], meta["g_col0"]
    g_lo, g_hi = meta["g_lo"], meta["g_hi"]
    TSUM, GTMAX = meta["TSUM"], meta["GTMAX"]
    b1_zero = meta.get("b1_zero", False)
    b2_zero = meta.get("b2_zero", False)

    GB = int(os.environ.get("GB", "3"))
    XB = int(os.environ.get("XB", "3"))
    nc = bacc.Bacc("TRN2", target_bir_lowering=False, debug=False,
                   num_devices=NC)
    f32, bf16, i16 = mybir.dt.float32, mybir.dt.bfloat16, mybir.dt.int16
    f8 = mybir.dt.float8e3

    x_d = nc.dram_tensor("x_pre", [B, 128, KPAD], f8, kind="ExternalInput")
    w1_d = nc.dram_tensor("w1", [128, KO, H], bf16, kind="ExternalInput")
    b1_d = nc.dram_tensor("b1r", [128, H], f32, kind="ExternalInput")
    b2_d = nc.dram_tensor("b2r", [128, O], f32, kind="ExternalInput")
    w2_d = nc.dram_tensor("w2", [128, 2, O], f32, kind="ExternalInput")
    iota_d = nc.dram_tensor("iota", [128, 128], bf16, kind="ExternalInput")
    idx_d = nc.dram_tensor("idx", [128, TSUM * 8], i16, kind="ExternalInput")
    dl_d = nc.dram_tensor("dstloc", [128, TSUM], f32, kind="ExternalInput")
    nm_d = nc.dram_tensor("normv", [128, TSUM], f32, kind="ExternalInput")
    d2_d = nc.dram_tensor("dis2", [128, B], f32, kind="ExternalInput")
    out_d = nc.dram_tensor("out", [PC, O], f32, kind="ExternalOutput")

    with tile.TileContext(nc) as tc:
        with tc.tile_pool(name="cons", bufs=1) as cons, \
             tc.tile_pool(name="xt", bufs=XB) as xtp, \
             tc.tile_pool(name="gp", bufs=GB) as gp, \
             tc.tile_pool(name="mp", bufs=4) as mp, \
             tc.tile_pool(name="wk", bufs=3) as wk, \
             tc.tile_pool(name="dram", bufs=1, space="DRAM") as dram, \
             tc.tile_pool(name="pA", bufs=2, space="PSUM") as pA, \
             tc.tile_pool(name="pG", bufs=2, space="PSUM") as pG, \
             tc.tile_pool(name="pT", bufs=2, space="PSUM") as pT, \
             tc.tile_pool(name="pO", bufs=2, space="PSUM") as pO:

            # ---- constants
            w1_sb = cons.tile([128, KO, H], bf16)
            nc.sync.dma_start(w1_sb[:], w1_d.ap())
            b1_sb = cons.tile([128, H], f32)
            nc.sync.dma_start(b1_sb[:], b1_d.ap())
            b2_sb = cons.tile([128, O], f32)
            nc.sync.dma_start(b2_sb[:], b2_d.ap())
            w2_sb = cons.tile([128, 2, O], f32)
            nc.sync.dma_start(w2_sb[:], w2_d.ap())
            iota_sb = cons.tile([128, 128], bf16)
            nc.sync.dma_start(iota_sb[:], iota_d.ap())
            idx_sb = cons.tile([128, TSUM * 8], i16)
            nc.sync.dma_start(idx_sb[:], idx_d.ap())
            dl_sb = cons.tile([128, TSUM], f32)
            nc.sync.dma_start(dl_sb[:], dl_d.ap())
            nm_sb = cons.tile([128, TSUM], f32)
            nc.sync.dma_start(nm_sb[:], nm_d.ap())
            d2_sb = cons.tile([128, B], f32)
            nc.sync.dma_start(d2_sb[:], d2_d.ap())
            ident = cons.tile([128, 128], f32)
            make_identity(nc, ident[:])

            hR = dram.tile([PC, HPAD], f8)
            zR = dram.tile([PC, HPAD], f8)
            hfull = dram.tile([NPAD, HPAD], f8, addr_space="Shared")
            zfull = dram.tile([NPAD, HPAD], f8, addr_space="Shared")

            # ---- phase A: h' = (2x) @ W1  (fp8 x, bf16 W1, fp32 accum)
            for b in _scope(nc, "phaseA", range(B)):
                xt = xtp.tile([128, KO, 128], f8, tag="xt")
                nc.sync.dma_start(xt[:], x_d.ap()[b])
                ph = pA.tile([128, H], f32, tag="ph")
                for k in range(KO):
                    nc.tensor.matmul(ph[:], lhsT=xt[:, k, :], rhs=w1_sb[:, k, :],
                                     start=(k == 0), stop=(k == KO - 1))
                hblk = wk.tile([128, HPAD], f8, tag="hblk")
                nc.vector.tensor_copy(hblk[:, :H], ph[:])
                nc.sync.dma_start(hR[b * 128:(b + 1) * 128, :], hblk[:])

            # ---- AllGather h
            with nc.named_scope("AG_h"):
                if sim_mode:
                    nc.sync.dma_start(hfull[0:PC, :], hR[:])
                else:
                    nc.gpsimd.collective_compute(
                        "AllGather", mybir.AluOpType.bypass,
                        ins=[hR[:]], outs=[hfull[:]],
                        replica_groups=[list(range(NC))])

            # ---- aggregation: per group one lo+hi gather, per block matmuls
            def aggregate_group(table, local_tbl, gi, acc_pool, epilogue):
                grp = groups[gi]
                c0 = g_col0[gi]
                GLO, GHI = g_lo[gi], g_hi[gi]
                G = gp.tile([128, GTMAX, HPAD], f8, tag="G")
                if GLO:
                    nc.gpsimd.dma_gather(
                        G[:, :GLO, :], table[0:LO, :],
                        idx_sb[:, c0 * 8:(c0 + GLO) * 8],
                        GLO * 128, GLO * 128, HPAD, single_packet=SINGLE_PKT)
                if GHI:
                    nc.gpsimd.dma_gather(
                        G[:, GLO:GLO + GHI, :], table[HIB:NPAD, :],
                        idx_sb[:, (c0 + GLO) * 8:(c0 + GLO + GHI) * 8],
                        GHI * 128, GHI * 128, HPAD, single_packet=SINGLE_PKT)
                for b in grp:
                    nc.sync.dma_start(G[:, self_off[b] - c0, :],
                                      local_tbl[b * 128:(b + 1) * 128, :])
                for b in grp:
                    TLb, THb = T_LO[b], T_HI[b]
                    TT = TLb + THb + 1
                    acc = acc_pool.tile([128, H], f32, tag="acc")
                    for t in range(TT):
                        if t < TLb:
                            col = lo_off[b] + t
                        elif t < TLb + THb:
                            col = hi_off[b] + t - TLb
                        else:
                            col = self_off[b]
                        M = mp.tile([128, 128], bf16, tag="M")
                        nc.vector.tensor_scalar(
                            out=M[:], in0=iota_sb[:],
                            scalar1=dl_sb[:, col:col + 1],
                            scalar2=nm_sb[:, col:col + 1],
                            op0=mybir.AluOpType.is_equal,
                            op1=mybir.AluOpType.mult)
                        nc.tensor.matmul(acc[:], lhsT=M[:],
                                         rhs=G[:, col - c0, :H],
                                         start=(t == 0), stop=(t == TT - 1))
                    epilogue(b, acc)

            # ---- layer 1 epilogue: z' = relu(acc + selfT + 8*b1) (=8z)
            def l1_epilogue(b, acc):
                if not b1_zero:
                    zsum = wk.tile([128, H], f32, tag="zsum")
                    nc.vector.tensor_add(out=zsum[:], in0=acc[:], in1=b1_sb[:])
                    acc = zsum
                zblk = wk.tile([128, HPAD], f8, tag="zblk")
                nc.scalar.activation(zblk[:, :H], acc[:],
                                     mybir.ActivationFunctionType.Relu)
                nc.sync.dma_start(zR[b * 128:(b + 1) * 128, :], zblk[:])

            for gi in _scope(nc, "layer1", range(len(groups))):
                aggregate_group(hfull, hR, gi, pG, l1_epilogue)

            # ---- AllGather z
            with nc.named_scope("AG_z"):
                if sim_mode:
                    nc.sync.dma_start(zfull[0:PC, :], zR[:])
                else:
                    nc.gpsimd.collective_compute(
                        "AllGather", mybir.AluOpType.bypass,
                        ins=[zR[:]], outs=[zfull[:]],
                        replica_groups=[list(range(NC))])

            # ---- layer 2 epilogue: out = agg2 @ (W2/32) + b2
            def l2_epilogue(b, acc):
                agg2 = wk.tile([128, H], f32, tag="agg2")
                nc.vector.tensor_copy(agg2[:], acc[:])
                aggT = wk.tile([128, 2, 128], f32, tag="aggT")
                for kt, (k0, kw) in enumerate(((0, 128), (128, H - 128))):
                    pt = pT.tile([128, 128], f32, tag="pt")
                    nc.tensor.transpose(pt[:kw, :], agg2[:, k0:k0 + kw], ident[:])
                    nc.vector.tensor_copy(aggT[:kw, kt, :], pt[:kw, :])
                po = pO.tile([128, O], f32, tag="po")
                nc.tensor.matmul(po[:], lhsT=aggT[:, 0, :], rhs=w2_sb[:, 0, :],
                                 start=True, stop=False)
                nc.tensor.matmul(po[:], lhsT=aggT[:H - 128, 1, :],
                                 rhs=w2_sb[:H - 128, 1, :],
                                 start=False, stop=True)
                ob = wk.tile([128, O], f32, tag="ob")
                if b2_zero:
                    nc.vector.tensor_copy(ob[:], po[:])
                else:
                    nc.vector.tensor_add(out=ob[:], in0=po[:], in1=b2_sb[:])
                nc.sync.dma_start(out_d.ap()[b * 128:(b + 1) * 128, :], ob[:])

            for gi in _scope(nc, "layer2", range(len(groups))):
                aggregate_group(zfull, zR, gi, pG, l2_epilogue)

    nc.compile()
    return nc


_CACHE = {}


def build(inputs, gs=None):
    """Preprocess + build + compile; returns (nc, in_maps, meta)."""
    per_core, meta = _preprocess(**inputs, gs=gs)
    key = (meta["TSUM"], tuple(meta["T_LO"]), tuple(meta["T_HI"]),
           meta["b1_zero"], meta["b2_zero"])
    if key not in _CACHE:
        _CACHE[key] = _build_program(meta)
    return _CACHE[key], per_core, meta


def kernel(**inputs) -> np.ndarray:
    from concourse import bass_utils
    nc, per_core, meta = build(inputs)
    res = bass_utils.run_bass_kernel_spmd(nc, per_core, core_ids=list(range(NC)))
    out = np.concatenate([res.results[c]["out"] for c in range(NC)], axis=0)
    return np.ascontiguousarray(out[:N]).astype(np.float32)
